# revision 19
# baseline (speedup 1.0000x reference)
"""DNDT (deep neural decision tree) forward kernel for 8 Trainium2 NeuronCores.

Math (per batch row b of 16384):
  h[f,j]   = (x[b,f] * W[j] + bias[f,j]) / t,  W = [1..4], bias = cumsum([0,-sorted_cuts])
  bins     = softmax_j(h)                       # [6, 4]
  leaf     = kron(bins[0], ..., bins[5])        # [4096]
  out[b]   = leaf @ leaf_score                  # [10]

Device algorithm (pure data parallel, 2048 rows/core, batch-major layout
[128 partitions x 16 rows-per-partition]):
  * softmax shift via analytic bound g(x) = (x + 3*relu(x))/t (args <= 0);
    bias/t pulled out of the exp and kron-folded into the score matrix S2
    host-side.  Device exponent: H[...,0] = -3/t*relu(x), H[...,j] = x*j/t+H0.
  * E = exp(H) (ACT, f32); pp = pair-krons on DVE (bf16 out).  H/E/pp run
    per 8-slot half so group 0's kron starts one half earlier.
  * A2 kron in the swapped (cd,ab) layout so the 256-wide kron mul runs at
    DVE 2x_1p (all-bf16, packed innermost): Brep = pp23 replicated along ab
    (ACT copy, 1x), A2 = Brep * bcast(pp01) (DVE 2x).  S2 rows are permuted
    host-side to match.  Group 0 skips Brep (direct 1x mul) to shorten the
    first group's serial chain.
  * per 4-slot group: PE transposes A2 -> tp (PSUM bf16); at2 staged to SBUF
    by DVE tensor_copy (bf16 packed PSUM read dual-pumps) for group 0, ACT
    for groups 1-3; 8 matmuls contract 256 against S2 (176 cols = 10 class
    blocks + normalizer block, padded to 256 f32 PSUM so no bank crossing).
    A2/transpose for groups 1-3 are emitted right after group 0's chain so
    the PE streams transposes while waiting for at2 stages.
  * back half, all f32: D = cpp*p45 on DVE straight from PSUM.  Group 0 and
    the last group run reduce-16 + fast-reciprocal + finalize on DVE under
    a priority boost (shortest serial chain at the pipeline head and tail);
    middle groups fold 16->8 on Pool, reduce-8 on DVE, finalize on Pool.
  * tile pools for the tiny back-half tensors use bufs=4 so a group's
    finalize never waits for an earlier group's out-DMA to complete.
  * input DMAs split across sequencers (x on SP, ident on ACT, s2 halves on
    Pool and SP) so the parameter load doesn't slow the x transfer.
  * 10 junk matmuls on x keep the PE busy from the moment x lands until the
    first transpose, so the HAM grants full PE clock early; an idle gap
    resets the ramp and leaves the matmul stream at the slow p-state.
"""

import numpy as np

import concourse.bass as bass
import concourse.tile as tile
from concourse import bacc, mybir
from concourse.bass_utils import run_bass_kernel_spmd

N_CORES = 8
B = 16384
BC = B // N_CORES          # rows per core = 2048
P = 128                    # partitions
M = BC // P                # rows per partition = 16
NG = 4                     # back groups
QS = M // NG               # slots per group = 4
F32 = mybir.dt.float32
BF16 = mybir.dt.bfloat16
N_WARM = 10                # junk matmuls to warm the PE clock gate
NBLK = 11                  # 10 class blocks + 1 all-ones normalizer block
WID = NBLK * 16            # 176


def _build_nc(neg3invt):
    nc = bacc.Bacc("TRN2", target_bir_lowering=False, debug=False,
                   num_devices=N_CORES)
    xd = nc.dram_tensor("xc", [P, M * 6], F32, kind="ExternalInput")
    idd = nc.dram_tensor("id", [P, P], BF16, kind="ExternalInput")
    s2ad = nc.dram_tensor("s2a", [P, WID], BF16, kind="ExternalInput")
    s2bd = nc.dram_tensor("s2b", [P, WID], BF16, kind="ExternalInput")
    od = nc.dram_tensor("o", [P, M * 10], F32, kind="ExternalOutput")

    with tile.TileContext(nc) as tc:
        with tc.tile_pool(name="consts", bufs=1) as consts, \
             tc.tile_pool(name="work", bufs=2) as work, \
             tc.tile_pool(name="fin", bufs=4) as finp, \
             tc.tile_pool(name="brp", bufs=2) as brp, \
             tc.tile_pool(name="a2p", bufs=4) as a2p, \
             tc.tile_pool(name="atp", bufs=2) as atp, \
             tc.tile_pool(name="ps_t", bufs=2, space="PSUM") as ps_t, \
             tc.tile_pool(name="ps_c", bufs=2, space="PSUM") as ps_c:
            ov = od[:].rearrange("p (i c) -> p i c", i=M)
            # input DMAs spread over four sequencers/queues
            xc_st = consts.tile([P, M * 6], F32)
            nc.sync.dma_start(out=xc_st[:], in_=xd[:])
            ident = consts.tile([P, P], BF16)
            nc.scalar.dma_start(out=ident[:], in_=idd[:])
            s2_sb = consts.tile([P, 2, WID], BF16)
            nc.gpsimd.dma_start(out=s2_sb[:, 0, :], in_=s2ad[:])
            nc.sync.dma_start(out=s2_sb[:, 1, :], in_=s2bd[:])

            # HAM warm-up: junk matmuls on the otherwise idle PE while the
            # front (DMAs, DVE H/E/kron) runs.  fp32 (slow path) on purpose.
            for _ in range(N_WARM):
                wps = ps_t.tile([P, 4, P], F32, tag="tp")
                nc.tensor.matmul(wps[0:96, 0, 0:96],
                                 lhsT=xc_st[:, 0:96], rhs=xc_st[:, 0:96],
                                 start=True, stop=True)

            invt = -neg3invt / 3.0
            xv = xc_st[:, 0:M * 6].rearrange("p (i f) -> p i f", i=M)

            # --- front per 8-slot half: H, E, pp ---
            H = work.tile([P, M, 6, 4], F32, tag="H")
            E = work.tile([P, M, 6, 4], F32, tag="E")
            pp = work.tile([P, M, 3, 16], BF16, tag="pp")
            for hf in range(2):
                sl = slice(hf * 8, hf * 8 + 8)
                nc.vector.tensor_scalar(out=H[:, sl, :, 0], in0=xv[:, sl],
                                        scalar1=0.0, scalar2=neg3invt,
                                        op0=mybir.AluOpType.max,
                                        op1=mybir.AluOpType.mult)
                for j in range(1, 4):
                    nc.vector.scalar_tensor_tensor(
                        out=H[:, sl, :, j], in0=xv[:, sl],
                        scalar=float(j) * invt, in1=H[:, sl, :, 0],
                        op0=mybir.AluOpType.mult, op1=mybir.AluOpType.add)
                nc.scalar.activation(
                    E[:, sl].rearrange("p i f j -> p (i f j)"),
                    H[:, sl].rearrange("p i f j -> p (i f j)"),
                    mybir.ActivationFunctionType.Exp)
                E2 = E[:, sl].rearrange("p i (g t) j -> p i g t j", t=2)
                nc.vector.tensor_mul(
                    pp[:, sl].rearrange("p i f (a b) -> p i f a b", a=4),
                    E2[:, :, :, 0, :, None].broadcast_to((P, 8, 3, 4, 4)),
                    E2[:, :, :, 1, None, :].broadcast_to((P, 8, 3, 4, 4)))
            pp01 = pp[:, :, 0, :]
            pp23 = pp[:, :, 1, :]
            p45 = pp[:, :, 2, :]

            def emit_a2_tp(bg):
                base = bg * QS
                slp = slice(base, base + QS)
                A2 = a2p.tile([P, QS, 16, 16], BF16, tag="a2", name=f"A2{bg}")
                if bg == 0:
                    # fast lane: direct 1x kron, no Brep hop
                    nc.vector.tensor_mul(
                        A2[:],
                        pp23[:, slp, :, None].broadcast_to((P, QS, 16, 16)),
                        pp01[:, slp, None, :].broadcast_to((P, QS, 16, 16)))
                else:
                    Br = brp.tile([P, QS, 16, 16], BF16, tag="br",
                                  name=f"Br{bg}")
                    nc.scalar.copy(out=Br[:],
                                   in_=pp23[:, slp, :, None].broadcast_to(
                                       (P, QS, 16, 16)))
                    nc.vector.tensor_mul(
                        A2[:], Br[:],
                        pp01[:, slp, None, :].broadcast_to((P, QS, 16, 16)))
                A2v = A2[:].rearrange("p s c a -> p s (c a)")
                tp = ps_t.tile([P, 2 * QS, P], BF16, tag="tp", name=f"tp{bg}")
                for s in range(QS):
                    for k in range(2):
                        nc.tensor.transpose(tp[:, 2 * s + k, :],
                                            A2v[:, s, k * P:(k + 1) * P],
                                            ident[:])
                return tp

            def emit_back(bg, tp):
                base = bg * QS
                slp = slice(base, base + QS)
                first = bg == 0
                last = bg == NG - 1
                at2 = atp.tile([P, 2 * QS, P], BF16, tag="at", name=f"at{bg}")
                if first:
                    # bf16 packed PSUM->SBUF copy dual-pumps on DVE
                    nc.vector.tensor_copy(out=at2[:], in_=tp[:])
                else:
                    nc.scalar.copy(out=at2[:], in_=tp[:])
                if last:
                    # tail: per 2-slot half chain, top priority, so the very
                    # last rows take the shortest path through DVE
                    for h2 in range(2):
                        cpp = ps_c.tile([P, 2, 256], F32, tag="cph",
                                        name=f"cp{bg}_{h2}")
                        for s2i in range(2):
                            s = 2 * h2 + s2i
                            nc.tensor.matmul(cpp[:, s2i, 0:WID],
                                             lhsT=at2[:, 2 * s, :],
                                             rhs=s2_sb[:, 0, :], start=True,
                                             stop=False)
                            nc.tensor.matmul(cpp[:, s2i, 0:WID],
                                             lhsT=at2[:, 2 * s + 1, :],
                                             rhs=s2_sb[:, 1, :], start=False,
                                             stop=True)
                        cv = cpp[:, :, 0:WID].rearrange(
                            "p s (c v) -> p s c v", c=NBLK)
                        sl2 = slice(base + 2 * h2, base + 2 * h2 + 2)
                        with tc.high_priority(offset=20):
                            D4 = finp.tile([P, 2, NBLK, 16], F32, tag="Dl",
                                           name=f"D{bg}_{h2}")
                            nc.vector.tensor_mul(
                                D4[:], cv,
                                p45[:, sl2, None, :].broadcast_to(
                                    (P, 2, NBLK, 16)))
                            Og = finp.tile([P, 2, NBLK], F32, tag="Ol",
                                           name=f"Og{bg}_{h2}")
                            nc.vector.tensor_reduce(Og[:], D4[:],
                                                    axis=mybir.AxisListType.X,
                                                    op=mybir.AluOpType.add)
                            zr = finp.tile([P, 2, 1], F32, tag="zrl",
                                           name=f"zr{bg}_{h2}")
                            nc.vector.reciprocal_approx_fast(zr[:, :, 0],
                                                             Og[:, :, 10])
                            Of = finp.tile([P, 2, 10], F32, tag="Ofl",
                                           name=f"Of{bg}_{h2}")
                            nc.vector.tensor_mul(
                                Of[:], Og[:, :, 0:10],
                                zr[:].broadcast_to((P, 2, 10)))
                            row0 = base + 2 * h2
                            nc.sync.dma_start(out=ov[:, row0:row0 + 2, :],
                                              in_=Of[:])
                    return
                cpp = ps_c.tile([P, QS, 256], F32, tag="cp", name=f"cp{bg}")
                for s in range(QS):
                    nc.tensor.matmul(cpp[:, s, 0:WID], lhsT=at2[:, 2 * s, :],
                                     rhs=s2_sb[:, 0, :], start=True,
                                     stop=False)
                    nc.tensor.matmul(cpp[:, s, 0:WID],
                                     lhsT=at2[:, 2 * s + 1, :],
                                     rhs=s2_sb[:, 1, :], start=False,
                                     stop=True)
                cv = cpp[:, :, 0:WID].rearrange("p s (c v) -> p s c v",
                                                c=NBLK)
                Og = finp.tile([P, QS, NBLK], F32, tag="O", name=f"Og{bg}")
                if first:
                    # head: D + reduce-16 + finalize all on DVE under a
                    # priority boost -- fewest cross-engine hops
                    with tc.high_priority(offset=60):
                        D4 = finp.tile([P, QS, NBLK, 16], F32, tag="D",
                                       name=f"D{bg}")
                        nc.vector.tensor_mul(
                            D4[:], cv,
                            p45[:, slp, None, :].broadcast_to(
                                (P, QS, NBLK, 16)))
                        nc.vector.tensor_reduce(Og[:], D4[:],
                                                axis=mybir.AxisListType.X,
                                                op=mybir.AluOpType.add)
                        zr = finp.tile([P, QS, 1], F32, tag="zr",
                                       name=f"zr{bg}")
                        nc.vector.reciprocal_approx_fast(zr[:, :, 0],
                                                         Og[:, :, 10])
                        Of = finp.tile([P, QS, 10], F32, tag="Of",
                                       name=f"Of{bg}")
                        nc.gpsimd.tensor_mul(Of[:], Og[:, :, 0:10],
                                             zr[:].broadcast_to((P, QS, 10)))
                        nc.scalar.dma_start(out=ov[:, base:base + QS, :],
                                            in_=Of[:])
                else:
                    D4 = finp.tile([P, QS, NBLK, 16], F32, tag="D",
                                   name=f"D{bg}")
                    nc.vector.tensor_mul(
                        D4[:], cv,
                        p45[:, slp, None, :].broadcast_to((P, QS, NBLK, 16)))
                    Dh = finp.tile([P, QS, NBLK, 8], F32, tag="Dh",
                                   name=f"Dh{bg}")
                    nc.gpsimd.tensor_add(Dh[:], D4[:, :, :, 0:8],
                                         D4[:, :, :, 8:16])
                    Dq = finp.tile([P, QS, NBLK, 4], F32, tag="Dq",
                                   name=f"Dq{bg}")
                    nc.gpsimd.tensor_add(Dq[:], Dh[:, :, :, 0:4],
                                         Dh[:, :, :, 4:8])
                    with tc.high_priority(offset=20):
                        nc.vector.tensor_reduce(Og[:], Dq[:],
                                                axis=mybir.AxisListType.X,
                                                op=mybir.AluOpType.add)
                        zr = finp.tile([P, QS, 1], F32, tag="zr",
                                       name=f"zr{bg}")
                        nc.vector.reciprocal_approx_fast(zr[:, :, 0],
                                                         Og[:, :, 10])
                        Of = finp.tile([P, QS, 10], F32, tag="Of",
                                       name=f"Of{bg}")
                        nc.gpsimd.tensor_mul(Of[:], Og[:, :, 0:10],
                                             zr[:].broadcast_to((P, QS, 10)))
                        nc.scalar.dma_start(out=ov[:, base:base + QS, :],
                                            in_=Of[:])

            # group 0's full chain first (lowest latency to first output),
            # then the remaining A2/transposes (PE streams them while ACT
            # stages at2), then the remaining back halves in order.
            tp0 = emit_a2_tp(0)
            emit_back(0, tp0)
            tps = {bg: emit_a2_tp(bg) for bg in range(1, NG)}
            for bg in range(1, NG):
                emit_back(bg, tps[bg])
    nc.compile()
    return nc


def prep_inputs(x, cuts, leaf_score, temperature):
    """Host-side parameter prep (tiny). Returns (in_maps, invt)."""
    import ml_dtypes
    x = np.ascontiguousarray(np.asarray(x, dtype=np.float32))
    cuts = np.asarray(cuts, dtype=np.float32)
    leaf_score = np.asarray(leaf_score, dtype=np.float32)
    invt = 1.0 / float(np.asarray(temperature).reshape(-1)[0])

    sc = np.sort(cuts, axis=1)
    bias = np.cumsum(np.concatenate([np.zeros((6, 1), np.float64), -sc],
                                    axis=1, dtype=np.float64), axis=1)  # [6,4]
    ebt = np.exp(bias * invt)                                           # [6,4]
    c0123 = np.einsum('a,b,c,d->abcd', ebt[0], ebt[1], ebt[2],
                      ebt[3]).reshape(256)
    c45 = np.einsum('a,b->ab', ebt[4], ebt[5]).reshape(16)
    xs = x.reshape(N_CORES, P, M * 6)

    s2 = np.zeros((256, WID), np.float64)
    s2[:, :160] = leaf_score.reshape(256, 16, 10).transpose(0, 2, 1).reshape(
        256, 160)
    s2[:, 160:] = 1.0
    s2 = s2 * c0123[:, None] * np.tile(c45, NBLK)[None, :]
    # device A2 kron is in (cd, ab) order; permute rows to match
    s2 = s2.reshape(16, 16, WID).transpose(1, 0, 2).reshape(256, WID)
    s2 = s2.reshape(2, P, WID).astype(ml_dtypes.bfloat16)
    ident = np.eye(P, dtype=ml_dtypes.bfloat16)

    common = {"id": np.ascontiguousarray(ident),
              "s2a": np.ascontiguousarray(s2[0]),
              "s2b": np.ascontiguousarray(s2[1])}
    in_maps = [dict(common, xc=np.ascontiguousarray(xs[i]))
               for i in range(N_CORES)]
    return in_maps, invt


_CACHE = {}


def kernel(x, cuts, leaf_score, temperature):
    in_maps, invt = prep_inputs(x, cuts, leaf_score, temperature)
    key = ("nc", float(invt))
    if key not in _CACHE:
        _CACHE[key] = _build_nc(-3.0 * invt)
        _CACHE["nc"] = _CACHE[key]
    nc = _CACHE[key]
    res = run_bass_kernel_spmd(nc, in_maps, list(range(N_CORES))).results
    out = np.concatenate([r["o"].reshape(BC, 10) for r in res], axis=0)
    return out.astype(np.float32)


# revision 20
# speedup vs baseline: 1.0284x; 1.0284x over previous
"""DNDT (deep neural decision tree) forward kernel for 8 Trainium2 NeuronCores.

Math (per batch row b of 16384):
  h[f,j]   = (x[b,f] * W[j] + bias[f,j]) / t,  W = [1..4], bias = cumsum([0,-sorted_cuts])
  bins     = softmax_j(h)                       # [6, 4]
  leaf     = kron(bins[0], ..., bins[5])        # [4096]
  out[b]   = leaf @ leaf_score                  # [10]

Device algorithm (pure data parallel, 2048 rows/core, batch-major layout
[128 partitions x 16 rows-per-partition]):
  * softmax shift via analytic bound g(x) = (x + 3*relu(x))/t (args <= 0);
    bias/t pulled out of the exp and kron-folded into the score matrix S2
    host-side.  Device exponent: H[...,0] = -3/t*relu(x), H[...,j] = x*j/t+H0.
  * E = exp(H) (ACT, f32); pp = pair-krons on DVE (bf16 out).  H/E/pp run
    per 8-slot half so group 0's kron starts one half earlier.
  * A2 kron in the swapped (cd,ab) layout so the 256-wide kron mul runs at
    DVE 2x_1p (all-bf16, packed innermost): Brep = pp23 replicated along ab
    (ACT copy, 1x), A2 = Brep * bcast(pp01) (DVE 2x).  S2 rows are permuted
    host-side to match.  Group 0 skips Brep (direct 1x mul) to shorten the
    first group's serial chain.
  * per 4-slot group: PE transposes A2 -> tp (PSUM bf16); at2 staged to SBUF
    by DVE tensor_copy (bf16 packed PSUM read dual-pumps) for group 0, ACT
    for groups 1-3; 8 matmuls contract 256 against S2 (176 cols = 10 class
    blocks + normalizer block, padded to 256 f32 PSUM so no bank crossing).
    A2/transpose for groups 1-3 are emitted right after group 0's chain so
    the PE streams transposes while waiting for at2 stages.
  * back half, all f32: D = cpp*p45 on DVE straight from PSUM.  Group 0 and
    the last group run reduce-16 + fast-reciprocal + finalize on DVE under
    a priority boost (shortest serial chain at the pipeline head and tail);
    middle groups fold 16->8 on Pool, reduce-8 on DVE, finalize on Pool.
  * tile pools for the tiny back-half tensors use bufs=4 so a group's
    finalize never waits for an earlier group's out-DMA to complete.
  * input DMAs split across sequencers (x on SP, ident on ACT, s2 halves on
    Pool and SP) so the parameter load doesn't slow the x transfer.
  * 10 junk matmuls on x keep the PE busy from the moment x lands until the
    first transpose, so the HAM grants full PE clock early; an idle gap
    resets the ramp and leaves the matmul stream at the slow p-state.
"""

import numpy as np

import concourse.bass as bass
import concourse.tile as tile
from concourse import bacc, mybir
from concourse.bass_utils import run_bass_kernel_spmd

N_CORES = 8
B = 16384
BC = B // N_CORES          # rows per core = 2048
P = 128                    # partitions
M = BC // P                # rows per partition = 16
NG = 4                     # back groups
QS = M // NG               # slots per group = 4
F32 = mybir.dt.float32
BF16 = mybir.dt.bfloat16
N_WARM = 10                # junk matmuls to warm the PE clock gate
NBLK = 11                  # 10 class blocks + 1 all-ones normalizer block
WID = NBLK * 16            # 176


def _build_nc(neg3invt):
    nc = bacc.Bacc("TRN2", target_bir_lowering=False, debug=False,
                   num_devices=N_CORES)
    xd = nc.dram_tensor("xc", [P, M * 6], F32, kind="ExternalInput")
    idd = nc.dram_tensor("id", [P, P], BF16, kind="ExternalInput")
    s2ad = nc.dram_tensor("s2a", [P, WID], BF16, kind="ExternalInput")
    s2bd = nc.dram_tensor("s2b", [P, WID], BF16, kind="ExternalInput")
    od = nc.dram_tensor("o", [P, M * 10], F32, kind="ExternalOutput")

    with tile.TileContext(nc) as tc:
        with tc.tile_pool(name="consts", bufs=1) as consts, \
             tc.tile_pool(name="work", bufs=2) as work, \
             tc.tile_pool(name="fin", bufs=4) as finp, \
             tc.tile_pool(name="brp", bufs=2) as brp, \
             tc.tile_pool(name="a2p", bufs=4) as a2p, \
             tc.tile_pool(name="atp", bufs=2) as atp, \
             tc.tile_pool(name="ps_t", bufs=2, space="PSUM") as ps_t, \
             tc.tile_pool(name="ps_c", bufs=2, space="PSUM") as ps_c:
            ov = od[:].rearrange("p (i c) -> p i c", i=M)
            # input DMAs spread over four sequencers/queues
            xc_st = consts.tile([P, M * 6], F32)
            nc.sync.dma_start(out=xc_st[:], in_=xd[:])
            ident = consts.tile([P, P], BF16)
            nc.scalar.dma_start(out=ident[:], in_=idd[:])
            s2_sb = consts.tile([P, 2, WID], BF16)
            nc.gpsimd.dma_start(out=s2_sb[:, 0, :], in_=s2ad[:])
            nc.sync.dma_start(out=s2_sb[:, 1, :], in_=s2bd[:])

            # HAM warm-up: junk matmuls on the otherwise idle PE while the
            # front (DMAs, DVE H/E/kron) runs.  fp32 (slow path) on purpose.
            for _ in range(N_WARM):
                wps = ps_t.tile([P, 4, P], F32, tag="tp")
                nc.tensor.matmul(wps[0:96, 0, 0:96],
                                 lhsT=xc_st[:, 0:96], rhs=xc_st[:, 0:96],
                                 start=True, stop=True)

            invt = -neg3invt / 3.0
            xv = xc_st[:, 0:M * 6].rearrange("p (i f) -> p i f", i=M)

            # --- front per 8-slot half: H, E, pp ---
            H = work.tile([P, M, 6, 4], F32, tag="H")
            E = work.tile([P, M, 6, 4], F32, tag="E")
            pp = work.tile([P, M, 3, 16], BF16, tag="pp")
            for hf in range(2):
                sl = slice(hf * 8, hf * 8 + 8)
                nc.vector.tensor_scalar(out=H[:, sl, :, 0], in0=xv[:, sl],
                                        scalar1=0.0, scalar2=neg3invt,
                                        op0=mybir.AluOpType.max,
                                        op1=mybir.AluOpType.mult)
                for j in range(1, 4):
                    nc.vector.scalar_tensor_tensor(
                        out=H[:, sl, :, j], in0=xv[:, sl],
                        scalar=float(j) * invt, in1=H[:, sl, :, 0],
                        op0=mybir.AluOpType.mult, op1=mybir.AluOpType.add)
                nc.scalar.activation(
                    E[:, sl].rearrange("p i f j -> p (i f j)"),
                    H[:, sl].rearrange("p i f j -> p (i f j)"),
                    mybir.ActivationFunctionType.Exp)
                E2 = E[:, sl].rearrange("p i (g t) j -> p i g t j", t=2)
                nc.vector.tensor_mul(
                    pp[:, sl].rearrange("p i f (a b) -> p i f a b", a=4),
                    E2[:, :, :, 0, :, None].broadcast_to((P, 8, 3, 4, 4)),
                    E2[:, :, :, 1, None, :].broadcast_to((P, 8, 3, 4, 4)))
            pp01 = pp[:, :, 0, :]
            pp23 = pp[:, :, 1, :]
            p45 = pp[:, :, 2, :]

            def emit_a2_tp(bg):
                base = bg * QS
                slp = slice(base, base + QS)
                A2 = a2p.tile([P, QS, 16, 16], BF16, tag="a2", name=f"A2{bg}")
                if bg == 0:
                    # fast lane: direct 1x kron, no Brep hop
                    nc.vector.tensor_mul(
                        A2[:],
                        pp23[:, slp, :, None].broadcast_to((P, QS, 16, 16)),
                        pp01[:, slp, None, :].broadcast_to((P, QS, 16, 16)))
                else:
                    Br = brp.tile([P, QS, 16, 16], BF16, tag="br",
                                  name=f"Br{bg}")
                    nc.scalar.copy(out=Br[:],
                                   in_=pp23[:, slp, :, None].broadcast_to(
                                       (P, QS, 16, 16)))
                    nc.vector.tensor_mul(
                        A2[:], Br[:],
                        pp01[:, slp, None, :].broadcast_to((P, QS, 16, 16)))
                A2v = A2[:].rearrange("p s c a -> p s (c a)")
                tp = ps_t.tile([P, 2 * QS, P], BF16, tag="tp", name=f"tp{bg}")
                for s in range(QS):
                    for k in range(2):
                        nc.tensor.transpose(tp[:, 2 * s + k, :],
                                            A2v[:, s, k * P:(k + 1) * P],
                                            ident[:])
                return tp

            def emit_back(bg, tp):
                base = bg * QS
                slp = slice(base, base + QS)
                first = bg == 0
                last = bg == NG - 1
                at2 = atp.tile([P, 2 * QS, P], BF16, tag="at", name=f"at{bg}")
                if first:
                    # bf16 packed PSUM->SBUF copy dual-pumps on DVE
                    nc.vector.tensor_copy(out=at2[:], in_=tp[:])
                else:
                    nc.scalar.copy(out=at2[:], in_=tp[:])
                if last:
                    # tail: per 2-slot half chain, top priority, so the very
                    # last rows take the shortest path through DVE
                    for h2 in range(2):
                        cpp = ps_c.tile([P, 2, 256], F32, tag="cph",
                                        name=f"cp{bg}_{h2}")
                        for s2i in range(2):
                            s = 2 * h2 + s2i
                            nc.tensor.matmul(cpp[:, s2i, 0:WID],
                                             lhsT=at2[:, 2 * s, :],
                                             rhs=s2_sb[:, 0, :], start=True,
                                             stop=False)
                            nc.tensor.matmul(cpp[:, s2i, 0:WID],
                                             lhsT=at2[:, 2 * s + 1, :],
                                             rhs=s2_sb[:, 1, :], start=False,
                                             stop=True)
                        cv = cpp[:, :, 0:WID].rearrange(
                            "p s (c v) -> p s c v", c=NBLK)
                        sl2 = slice(base + 2 * h2, base + 2 * h2 + 2)
                        with tc.high_priority(offset=100):
                            D4 = finp.tile([P, 2, NBLK, 16], F32, tag="Dl",
                                           name=f"D{bg}_{h2}")
                            nc.vector.tensor_mul(
                                D4[:], cv,
                                p45[:, sl2, None, :].broadcast_to(
                                    (P, 2, NBLK, 16)))
                            Og = finp.tile([P, 2, NBLK], F32, tag="Ol",
                                           name=f"Og{bg}_{h2}")
                            nc.vector.tensor_reduce(Og[:], D4[:],
                                                    axis=mybir.AxisListType.X,
                                                    op=mybir.AluOpType.add)
                            zr = finp.tile([P, 2, 1], F32, tag="zrl",
                                           name=f"zr{bg}_{h2}")
                            nc.vector.reciprocal_approx_fast(zr[:, :, 0],
                                                             Og[:, :, 10])
                            Of = finp.tile([P, 2, 10], F32, tag="Ofl",
                                           name=f"Of{bg}_{h2}")
                            nc.vector.tensor_mul(
                                Of[:], Og[:, :, 0:10],
                                zr[:].broadcast_to((P, 2, 10)))
                            row0 = base + 2 * h2
                            nc.sync.dma_start(out=ov[:, row0:row0 + 2, :],
                                              in_=Of[:])
                    return
                cpp = ps_c.tile([P, QS, 256], F32, tag="cp", name=f"cp{bg}")
                for s in range(QS):
                    nc.tensor.matmul(cpp[:, s, 0:WID], lhsT=at2[:, 2 * s, :],
                                     rhs=s2_sb[:, 0, :], start=True,
                                     stop=False)
                    nc.tensor.matmul(cpp[:, s, 0:WID],
                                     lhsT=at2[:, 2 * s + 1, :],
                                     rhs=s2_sb[:, 1, :], start=False,
                                     stop=True)
                cv = cpp[:, :, 0:WID].rearrange("p s (c v) -> p s c v",
                                                c=NBLK)
                Og = finp.tile([P, QS, NBLK], F32, tag="O", name=f"Og{bg}")
                if first:
                    # head: D + reduce-16 + finalize all on DVE under a
                    # priority boost -- fewest cross-engine hops
                    with tc.high_priority(offset=60):
                        D4 = finp.tile([P, QS, NBLK, 16], F32, tag="D",
                                       name=f"D{bg}")
                        nc.vector.tensor_mul(
                            D4[:], cv,
                            p45[:, slp, None, :].broadcast_to(
                                (P, QS, NBLK, 16)))
                        nc.vector.tensor_reduce(Og[:], D4[:],
                                                axis=mybir.AxisListType.X,
                                                op=mybir.AluOpType.add)
                        zr = finp.tile([P, QS, 1], F32, tag="zr",
                                       name=f"zr{bg}")
                        nc.vector.reciprocal_approx_fast(zr[:, :, 0],
                                                         Og[:, :, 10])
                        Of = finp.tile([P, QS, 10], F32, tag="Of",
                                       name=f"Of{bg}")
                        nc.gpsimd.tensor_mul(Of[:], Og[:, :, 0:10],
                                             zr[:].broadcast_to((P, QS, 10)))
                        nc.scalar.dma_start(out=ov[:, base:base + QS, :],
                                            in_=Of[:])
                else:
                    D4 = finp.tile([P, QS, NBLK, 16], F32, tag="D",
                                   name=f"D{bg}")
                    nc.vector.tensor_mul(
                        D4[:], cv,
                        p45[:, slp, None, :].broadcast_to((P, QS, NBLK, 16)))
                    Dh = finp.tile([P, QS, NBLK, 8], F32, tag="Dh",
                                   name=f"Dh{bg}")
                    nc.gpsimd.tensor_add(Dh[:], D4[:, :, :, 0:8],
                                         D4[:, :, :, 8:16])
                    Dq = finp.tile([P, QS, NBLK, 4], F32, tag="Dq",
                                   name=f"Dq{bg}")
                    nc.gpsimd.tensor_add(Dq[:], Dh[:, :, :, 0:4],
                                         Dh[:, :, :, 4:8])
                    with tc.high_priority(offset=20):
                        nc.vector.tensor_reduce(Og[:], Dq[:],
                                                axis=mybir.AxisListType.X,
                                                op=mybir.AluOpType.add)
                        zr = finp.tile([P, QS, 1], F32, tag="zr",
                                       name=f"zr{bg}")
                        nc.vector.reciprocal_approx_fast(zr[:, :, 0],
                                                         Og[:, :, 10])
                        Of = finp.tile([P, QS, 10], F32, tag="Of",
                                       name=f"Of{bg}")
                        nc.gpsimd.tensor_mul(Of[:], Og[:, :, 0:10],
                                             zr[:].broadcast_to((P, QS, 10)))
                        nc.scalar.dma_start(out=ov[:, base:base + QS, :],
                                            in_=Of[:])

            # group 0's full chain first (lowest latency to first output),
            # then the remaining A2/transposes (PE streams them while ACT
            # stages at2), then the remaining back halves in order.
            tp0 = emit_a2_tp(0)
            emit_back(0, tp0)
            tps = {bg: emit_a2_tp(bg) for bg in range(1, NG)}
            for bg in range(1, NG):
                emit_back(bg, tps[bg])
    nc.compile()
    return nc


def prep_inputs(x, cuts, leaf_score, temperature):
    """Host-side parameter prep (tiny). Returns (in_maps, invt)."""
    import ml_dtypes
    x = np.ascontiguousarray(np.asarray(x, dtype=np.float32))
    cuts = np.asarray(cuts, dtype=np.float32)
    leaf_score = np.asarray(leaf_score, dtype=np.float32)
    invt = 1.0 / float(np.asarray(temperature).reshape(-1)[0])

    sc = np.sort(cuts, axis=1)
    bias = np.cumsum(np.concatenate([np.zeros((6, 1), np.float64), -sc],
                                    axis=1, dtype=np.float64), axis=1)  # [6,4]
    ebt = np.exp(bias * invt)                                           # [6,4]
    c0123 = np.einsum('a,b,c,d->abcd', ebt[0], ebt[1], ebt[2],
                      ebt[3]).reshape(256)
    c45 = np.einsum('a,b->ab', ebt[4], ebt[5]).reshape(16)
    xs = x.reshape(N_CORES, P, M * 6)

    s2 = np.zeros((256, WID), np.float64)
    s2[:, :160] = leaf_score.reshape(256, 16, 10).transpose(0, 2, 1).reshape(
        256, 160)
    s2[:, 160:] = 1.0
    s2 = s2 * c0123[:, None] * np.tile(c45, NBLK)[None, :]
    # device A2 kron is in (cd, ab) order; permute rows to match
    s2 = s2.reshape(16, 16, WID).transpose(1, 0, 2).reshape(256, WID)
    s2 = s2.reshape(2, P, WID).astype(ml_dtypes.bfloat16)
    ident = np.eye(P, dtype=ml_dtypes.bfloat16)

    common = {"id": np.ascontiguousarray(ident),
              "s2a": np.ascontiguousarray(s2[0]),
              "s2b": np.ascontiguousarray(s2[1])}
    in_maps = [dict(common, xc=np.ascontiguousarray(xs[i]))
               for i in range(N_CORES)]
    return in_maps, invt


_CACHE = {}


def kernel(x, cuts, leaf_score, temperature):
    in_maps, invt = prep_inputs(x, cuts, leaf_score, temperature)
    key = ("nc", float(invt))
    if key not in _CACHE:
        _CACHE[key] = _build_nc(-3.0 * invt)
        _CACHE["nc"] = _CACHE[key]
    nc = _CACHE[key]
    res = run_bass_kernel_spmd(nc, in_maps, list(range(N_CORES))).results
    out = np.concatenate([r["o"].reshape(BC, 10) for r in res], axis=0)
    return out.astype(np.float32)


# revision 21
# speedup vs baseline: 1.0412x; 1.0125x over previous
"""DNDT (deep neural decision tree) forward kernel for 8 Trainium2 NeuronCores.

Math (per batch row b of 16384):
  h[f,j]   = (x[b,f] * W[j] + bias[f,j]) / t,  W = [1..4], bias = cumsum([0,-sorted_cuts])
  bins     = softmax_j(h)                       # [6, 4]
  leaf     = kron(bins[0], ..., bins[5])        # [4096]
  out[b]   = leaf @ leaf_score                  # [10]

Device algorithm (pure data parallel, 2048 rows/core, batch-major layout
[128 partitions x 16 rows-per-partition]):
  * softmax shift via analytic bound g(x) = (x + 3*relu(x))/t (args <= 0);
    bias/t pulled out of the exp and kron-folded into the score matrix S2
    host-side.  Device exponent: H[...,0] = -3/t*relu(x), H[...,j] = x*j/t+H0.
  * E = exp(H) (ACT, f32); pp = pair-krons on DVE (bf16 out).  H/E/pp run
    per 8-slot half so group 0's kron starts one half earlier.
  * A2 kron in the swapped (cd,ab) layout so the 256-wide kron mul runs at
    DVE 2x_1p (all-bf16, packed innermost): Brep = pp23 replicated along ab
    (ACT copy, 1x), A2 = Brep * bcast(pp01) (DVE 2x).  S2 rows are permuted
    host-side to match.  Group 0 skips Brep (direct 1x mul) to shorten the
    first group's serial chain.
  * per 4-slot group: PE transposes A2 -> tp (PSUM bf16); at2 staged to SBUF
    by DVE tensor_copy (bf16 packed PSUM read dual-pumps) for group 0, ACT
    for groups 1-3; 8 matmuls contract 256 against S2 (176 cols = 10 class
    blocks + normalizer block, padded to 256 f32 PSUM so no bank crossing).
    A2/transpose for groups 1-3 are emitted right after group 0's chain so
    the PE streams transposes while waiting for at2 stages.
  * back half, all f32: D = cpp*p45 on DVE straight from PSUM.  Group 0 and
    the last group run reduce-16 + fast-reciprocal + finalize on DVE under
    a priority boost (shortest serial chain at the pipeline head and tail);
    middle groups fold 16->8 on Pool, reduce-8 on DVE, finalize on Pool.
  * tile pools for the tiny back-half tensors use bufs=4 so a group's
    finalize never waits for an earlier group's out-DMA to complete.
  * input DMAs split across sequencers (x on SP, ident on ACT, s2 halves on
    Pool and SP) so the parameter load doesn't slow the x transfer.
  * 10 junk matmuls on x keep the PE busy from the moment x lands until the
    first transpose, so the HAM grants full PE clock early; an idle gap
    resets the ramp and leaves the matmul stream at the slow p-state.
"""

import numpy as np

import concourse.bass as bass
import concourse.tile as tile
from concourse import bacc, mybir
from concourse.bass_utils import run_bass_kernel_spmd

N_CORES = 8
B = 16384
BC = B // N_CORES          # rows per core = 2048
P = 128                    # partitions
M = BC // P                # rows per partition = 16
NG = 4                     # back groups
QS = M // NG               # slots per group = 4
F32 = mybir.dt.float32
BF16 = mybir.dt.bfloat16
N_WARM = 10                # junk matmuls to warm the PE clock gate
NBLK = 11                  # 10 class blocks + 1 all-ones normalizer block
WID = NBLK * 16            # 176


def _build_nc(neg3invt):
    nc = bacc.Bacc("TRN2", target_bir_lowering=False, debug=False,
                   num_devices=N_CORES)
    xd = nc.dram_tensor("xc", [P, M * 6], F32, kind="ExternalInput")
    idd = nc.dram_tensor("id", [P, P], BF16, kind="ExternalInput")
    s2ad = nc.dram_tensor("s2a", [P, WID], BF16, kind="ExternalInput")
    s2bd = nc.dram_tensor("s2b", [P, WID], BF16, kind="ExternalInput")
    od = nc.dram_tensor("o", [P, M * 10], F32, kind="ExternalOutput")

    with tile.TileContext(nc) as tc:
        with tc.tile_pool(name="consts", bufs=1) as consts, \
             tc.tile_pool(name="work", bufs=2) as work, \
             tc.tile_pool(name="fin", bufs=4) as finp, \
             tc.tile_pool(name="brp", bufs=2) as brp, \
             tc.tile_pool(name="a2p", bufs=4) as a2p, \
             tc.tile_pool(name="atp", bufs=2) as atp, \
             tc.tile_pool(name="ps_t", bufs=2, space="PSUM") as ps_t, \
             tc.tile_pool(name="ps_c", bufs=2, space="PSUM") as ps_c:
            ov = od[:].rearrange("p (i c) -> p i c", i=M)
            # input DMAs spread over four sequencers/queues
            xc_st = consts.tile([P, M * 6], F32)
            nc.sync.dma_start(out=xc_st[:], in_=xd[:])
            ident = consts.tile([P, P], BF16)
            nc.scalar.dma_start(out=ident[:], in_=idd[:])
            s2_sb = consts.tile([P, 2, WID], BF16)
            nc.gpsimd.dma_start(out=s2_sb[:, 0, :], in_=s2ad[:])
            nc.gpsimd.dma_start(out=s2_sb[:, 1, :], in_=s2bd[:])

            # HAM warm-up: junk matmuls on the otherwise idle PE while the
            # front (DMAs, DVE H/E/kron) runs.  fp32 (slow path) on purpose.
            for _ in range(N_WARM):
                wps = ps_t.tile([P, 4, P], F32, tag="tp")
                nc.tensor.matmul(wps[0:96, 0, 0:96],
                                 lhsT=xc_st[:, 0:96], rhs=xc_st[:, 0:96],
                                 start=True, stop=True)

            invt = -neg3invt / 3.0
            xv = xc_st[:, 0:M * 6].rearrange("p (i f) -> p i f", i=M)

            # --- front per 8-slot half: H, E, pp ---
            H = work.tile([P, M, 6, 4], F32, tag="H")
            E = work.tile([P, M, 6, 4], F32, tag="E")
            pp = work.tile([P, M, 3, 16], BF16, tag="pp")
            for hf in range(2):
                sl = slice(hf * 8, hf * 8 + 8)
                nc.vector.tensor_scalar(out=H[:, sl, :, 0], in0=xv[:, sl],
                                        scalar1=0.0, scalar2=neg3invt,
                                        op0=mybir.AluOpType.max,
                                        op1=mybir.AluOpType.mult)
                for j in range(1, 4):
                    nc.vector.scalar_tensor_tensor(
                        out=H[:, sl, :, j], in0=xv[:, sl],
                        scalar=float(j) * invt, in1=H[:, sl, :, 0],
                        op0=mybir.AluOpType.mult, op1=mybir.AluOpType.add)
                nc.scalar.activation(
                    E[:, sl].rearrange("p i f j -> p (i f j)"),
                    H[:, sl].rearrange("p i f j -> p (i f j)"),
                    mybir.ActivationFunctionType.Exp)
                E2 = E[:, sl].rearrange("p i (g t) j -> p i g t j", t=2)
                nc.vector.tensor_mul(
                    pp[:, sl].rearrange("p i f (a b) -> p i f a b", a=4),
                    E2[:, :, :, 0, :, None].broadcast_to((P, 8, 3, 4, 4)),
                    E2[:, :, :, 1, None, :].broadcast_to((P, 8, 3, 4, 4)))
            pp01 = pp[:, :, 0, :]
            pp23 = pp[:, :, 1, :]
            p45 = pp[:, :, 2, :]

            def emit_a2_tp(bg):
                base = bg * QS
                slp = slice(base, base + QS)
                A2 = a2p.tile([P, QS, 16, 16], BF16, tag="a2", name=f"A2{bg}")
                if bg == 0:
                    # fast lane: direct 1x kron, no Brep hop
                    nc.vector.tensor_mul(
                        A2[:],
                        pp23[:, slp, :, None].broadcast_to((P, QS, 16, 16)),
                        pp01[:, slp, None, :].broadcast_to((P, QS, 16, 16)))
                else:
                    Br = brp.tile([P, QS, 16, 16], BF16, tag="br",
                                  name=f"Br{bg}")
                    nc.scalar.copy(out=Br[:],
                                   in_=pp23[:, slp, :, None].broadcast_to(
                                       (P, QS, 16, 16)))
                    nc.vector.tensor_mul(
                        A2[:], Br[:],
                        pp01[:, slp, None, :].broadcast_to((P, QS, 16, 16)))
                A2v = A2[:].rearrange("p s c a -> p s (c a)")
                tp = ps_t.tile([P, 2 * QS, P], BF16, tag="tp", name=f"tp{bg}")
                for s in range(QS):
                    for k in range(2):
                        nc.tensor.transpose(tp[:, 2 * s + k, :],
                                            A2v[:, s, k * P:(k + 1) * P],
                                            ident[:])
                return tp

            def emit_back(bg, tp):
                base = bg * QS
                slp = slice(base, base + QS)
                first = bg == 0
                last = bg == NG - 1
                at2 = atp.tile([P, 2 * QS, P], BF16, tag="at", name=f"at{bg}")
                if first:
                    # bf16 packed PSUM->SBUF copy dual-pumps on DVE
                    nc.vector.tensor_copy(out=at2[:], in_=tp[:])
                else:
                    nc.scalar.copy(out=at2[:], in_=tp[:])
                if last:
                    # tail: per 2-slot half chain, top priority, so the very
                    # last rows take the shortest path through DVE
                    for h2 in range(2):
                        cpp = ps_c.tile([P, 2, 256], F32, tag="cph",
                                        name=f"cp{bg}_{h2}")
                        for s2i in range(2):
                            s = 2 * h2 + s2i
                            nc.tensor.matmul(cpp[:, s2i, 0:WID],
                                             lhsT=at2[:, 2 * s, :],
                                             rhs=s2_sb[:, 0, :], start=True,
                                             stop=False)
                            nc.tensor.matmul(cpp[:, s2i, 0:WID],
                                             lhsT=at2[:, 2 * s + 1, :],
                                             rhs=s2_sb[:, 1, :], start=False,
                                             stop=True)
                        cv = cpp[:, :, 0:WID].rearrange(
                            "p s (c v) -> p s c v", c=NBLK)
                        sl2 = slice(base + 2 * h2, base + 2 * h2 + 2)
                        with tc.high_priority(offset=100):
                            D4 = finp.tile([P, 2, NBLK, 16], F32, tag="Dl",
                                           name=f"D{bg}_{h2}")
                            nc.vector.tensor_mul(
                                D4[:], cv,
                                p45[:, sl2, None, :].broadcast_to(
                                    (P, 2, NBLK, 16)))
                            Og = finp.tile([P, 2, NBLK], F32, tag="Ol",
                                           name=f"Og{bg}_{h2}")
                            nc.vector.tensor_reduce(Og[:], D4[:],
                                                    axis=mybir.AxisListType.X,
                                                    op=mybir.AluOpType.add)
                            zr = finp.tile([P, 2, 1], F32, tag="zrl",
                                           name=f"zr{bg}_{h2}")
                            nc.vector.reciprocal_approx_fast(zr[:, :, 0],
                                                             Og[:, :, 10])
                            Of = finp.tile([P, 2, 10], F32, tag="Ofl",
                                           name=f"Of{bg}_{h2}")
                            nc.vector.tensor_mul(
                                Of[:], Og[:, :, 0:10],
                                zr[:].broadcast_to((P, 2, 10)))
                            row0 = base + 2 * h2
                            nc.sync.dma_start(out=ov[:, row0:row0 + 2, :],
                                              in_=Of[:])
                    return
                cpp = ps_c.tile([P, QS, 256], F32, tag="cp", name=f"cp{bg}")
                for s in range(QS):
                    nc.tensor.matmul(cpp[:, s, 0:WID], lhsT=at2[:, 2 * s, :],
                                     rhs=s2_sb[:, 0, :], start=True,
                                     stop=False)
                    nc.tensor.matmul(cpp[:, s, 0:WID],
                                     lhsT=at2[:, 2 * s + 1, :],
                                     rhs=s2_sb[:, 1, :], start=False,
                                     stop=True)
                cv = cpp[:, :, 0:WID].rearrange("p s (c v) -> p s c v",
                                                c=NBLK)
                Og = finp.tile([P, QS, NBLK], F32, tag="O", name=f"Og{bg}")
                if first:
                    # head: D + reduce-16 + finalize all on DVE under a
                    # priority boost -- fewest cross-engine hops
                    with tc.high_priority(offset=60):
                        D4 = finp.tile([P, QS, NBLK, 16], F32, tag="D",
                                       name=f"D{bg}")
                        nc.vector.tensor_mul(
                            D4[:], cv,
                            p45[:, slp, None, :].broadcast_to(
                                (P, QS, NBLK, 16)))
                        nc.vector.tensor_reduce(Og[:], D4[:],
                                                axis=mybir.AxisListType.X,
                                                op=mybir.AluOpType.add)
                        zr = finp.tile([P, QS, 1], F32, tag="zr",
                                       name=f"zr{bg}")
                        nc.vector.reciprocal_approx_fast(zr[:, :, 0],
                                                         Og[:, :, 10])
                        Of = finp.tile([P, QS, 10], F32, tag="Of",
                                       name=f"Of{bg}")
                        nc.gpsimd.tensor_mul(Of[:], Og[:, :, 0:10],
                                             zr[:].broadcast_to((P, QS, 10)))
                        nc.scalar.dma_start(out=ov[:, base:base + QS, :],
                                            in_=Of[:])
                else:
                    D4 = finp.tile([P, QS, NBLK, 16], F32, tag="D",
                                   name=f"D{bg}")
                    nc.vector.tensor_mul(
                        D4[:], cv,
                        p45[:, slp, None, :].broadcast_to((P, QS, NBLK, 16)))
                    Dh = finp.tile([P, QS, NBLK, 8], F32, tag="Dh",
                                   name=f"Dh{bg}")
                    nc.gpsimd.tensor_add(Dh[:], D4[:, :, :, 0:8],
                                         D4[:, :, :, 8:16])
                    if bg == 1:
                        Dq = finp.tile([P, QS, NBLK, 4], F32, tag="Dq",
                                       name=f"Dq{bg}")
                        nc.gpsimd.tensor_add(Dq[:], Dh[:, :, :, 0:4],
                                             Dh[:, :, :, 4:8])
                        red_in = Dq
                    else:
                        red_in = Dh
                    with tc.high_priority(offset=20):
                        nc.vector.tensor_reduce(Og[:], red_in[:],
                                                axis=mybir.AxisListType.X,
                                                op=mybir.AluOpType.add)
                        zr = finp.tile([P, QS, 1], F32, tag="zr",
                                       name=f"zr{bg}")
                        nc.vector.reciprocal_approx_fast(zr[:, :, 0],
                                                         Og[:, :, 10])
                        Of = finp.tile([P, QS, 10], F32, tag="Of",
                                       name=f"Of{bg}")
                        nc.gpsimd.tensor_mul(Of[:], Og[:, :, 0:10],
                                             zr[:].broadcast_to((P, QS, 10)))
                        nc.scalar.dma_start(out=ov[:, base:base + QS, :],
                                            in_=Of[:])

            # group 0's full chain first (lowest latency to first output),
            # then the remaining A2/transposes (PE streams them while ACT
            # stages at2), then the remaining back halves in order.
            tp0 = emit_a2_tp(0)
            emit_back(0, tp0)
            tps = {bg: emit_a2_tp(bg) for bg in range(1, NG)}
            for bg in range(1, NG):
                emit_back(bg, tps[bg])
    nc.compile()
    return nc


def prep_inputs(x, cuts, leaf_score, temperature):
    """Host-side parameter prep (tiny). Returns (in_maps, invt)."""
    import ml_dtypes
    x = np.ascontiguousarray(np.asarray(x, dtype=np.float32))
    cuts = np.asarray(cuts, dtype=np.float32)
    leaf_score = np.asarray(leaf_score, dtype=np.float32)
    invt = 1.0 / float(np.asarray(temperature).reshape(-1)[0])

    sc = np.sort(cuts, axis=1)
    bias = np.cumsum(np.concatenate([np.zeros((6, 1), np.float64), -sc],
                                    axis=1, dtype=np.float64), axis=1)  # [6,4]
    ebt = np.exp(bias * invt)                                           # [6,4]
    c0123 = np.einsum('a,b,c,d->abcd', ebt[0], ebt[1], ebt[2],
                      ebt[3]).reshape(256)
    c45 = np.einsum('a,b->ab', ebt[4], ebt[5]).reshape(16)
    xs = x.reshape(N_CORES, P, M * 6)

    s2 = np.zeros((256, WID), np.float64)
    s2[:, :160] = leaf_score.reshape(256, 16, 10).transpose(0, 2, 1).reshape(
        256, 160)
    s2[:, 160:] = 1.0
    s2 = s2 * c0123[:, None] * np.tile(c45, NBLK)[None, :]
    # device A2 kron is in (cd, ab) order; permute rows to match
    s2 = s2.reshape(16, 16, WID).transpose(1, 0, 2).reshape(256, WID)
    s2 = s2.reshape(2, P, WID).astype(ml_dtypes.bfloat16)
    ident = np.eye(P, dtype=ml_dtypes.bfloat16)

    common = {"id": np.ascontiguousarray(ident),
              "s2a": np.ascontiguousarray(s2[0]),
              "s2b": np.ascontiguousarray(s2[1])}
    in_maps = [dict(common, xc=np.ascontiguousarray(xs[i]))
               for i in range(N_CORES)]
    return in_maps, invt


_CACHE = {}


def kernel(x, cuts, leaf_score, temperature):
    in_maps, invt = prep_inputs(x, cuts, leaf_score, temperature)
    key = ("nc", float(invt))
    if key not in _CACHE:
        _CACHE[key] = _build_nc(-3.0 * invt)
        _CACHE["nc"] = _CACHE[key]
    nc = _CACHE[key]
    res = run_bass_kernel_spmd(nc, in_maps, list(range(N_CORES))).results
    out = np.concatenate([r["o"].reshape(BC, 10) for r in res], axis=0)
    return out.astype(np.float32)


# revision 22
# speedup vs baseline: 1.0566x; 1.0148x over previous
"""DNDT (deep neural decision tree) forward kernel for 8 Trainium2 NeuronCores.

Math (per batch row b of 16384):
  h[f,j]   = (x[b,f] * W[j] + bias[f,j]) / t,  W = [1..4], bias = cumsum([0,-sorted_cuts])
  bins     = softmax_j(h)                       # [6, 4]
  leaf     = kron(bins[0], ..., bins[5])        # [4096]
  out[b]   = leaf @ leaf_score                  # [10]

Device algorithm (pure data parallel, 2048 rows/core, batch-major layout
[128 partitions x 16 rows-per-partition]):
  * softmax shift via analytic bound g(x) = (x + 3*relu(x))/t (args <= 0);
    bias/t pulled out of the exp and kron-folded into the score matrix S2
    host-side.  Device exponent: H[...,0] = -3/t*relu(x), H[...,j] = x*j/t+H0.
  * E = exp(H) (ACT, f32); pp = pair-krons on DVE (bf16 out).  H/E/pp run
    per 8-slot half so group 0's kron starts one half earlier.
  * A2 kron in the swapped (cd,ab) layout so the 256-wide kron mul runs at
    DVE 2x_1p (all-bf16, packed innermost): Brep = pp23 replicated along ab
    (ACT copy, 1x), A2 = Brep * bcast(pp01) (DVE 2x).  S2 rows are permuted
    host-side to match.  Group 0 skips Brep (direct 1x mul) to shorten the
    first group's serial chain.
  * per 4-slot group: PE transposes A2 -> tp (PSUM bf16); at2 staged to SBUF
    by DVE tensor_copy (bf16 packed PSUM read dual-pumps) for group 0, ACT
    for groups 1-3; 8 matmuls contract 256 against S2 (176 cols = 10 class
    blocks + normalizer block, padded to 256 f32 PSUM so no bank crossing).
    A2/transpose for groups 1-3 are emitted right after group 0's chain so
    the PE streams transposes while waiting for at2 stages.
  * back half, all f32: D = cpp*p45 on DVE straight from PSUM.  Group 0 and
    the last group run reduce-16 + fast-reciprocal + finalize on DVE under
    a priority boost (shortest serial chain at the pipeline head and tail);
    middle groups fold 16->8 on Pool, reduce-8 on DVE, finalize on Pool.
  * tile pools for the tiny back-half tensors use bufs=4 so a group's
    finalize never waits for an earlier group's out-DMA to complete.
  * input DMAs split across sequencers (x on SP, ident on ACT, s2 halves on
    Pool and SP) so the parameter load doesn't slow the x transfer.
  * 10 junk matmuls on x keep the PE busy from the moment x lands until the
    first transpose, so the HAM grants full PE clock early; an idle gap
    resets the ramp and leaves the matmul stream at the slow p-state.
"""

import numpy as np

import concourse.bass as bass
import concourse.tile as tile
from concourse import bacc, mybir
from concourse.bass_utils import run_bass_kernel_spmd

N_CORES = 8
B = 16384
BC = B // N_CORES          # rows per core = 2048
P = 128                    # partitions
M = BC // P                # rows per partition = 16
NG = 4                     # back groups
QS = M // NG               # slots per group = 4
F32 = mybir.dt.float32
BF16 = mybir.dt.bfloat16
N_WARM = 10                # junk matmuls to warm the PE clock gate
NBLK = 11                  # 10 class blocks + 1 all-ones normalizer block
WID = NBLK * 16            # 176


def _build_nc(neg3invt):
    nc = bacc.Bacc("TRN2", target_bir_lowering=False, debug=False,
                   num_devices=N_CORES)
    xd = nc.dram_tensor("xc", [P, M * 6], F32, kind="ExternalInput")
    idd = nc.dram_tensor("id", [P, P], BF16, kind="ExternalInput")
    s2ad = nc.dram_tensor("s2a", [P, WID], BF16, kind="ExternalInput")
    s2bd = nc.dram_tensor("s2b", [P, WID], BF16, kind="ExternalInput")
    od = nc.dram_tensor("o", [P, M * 10], F32, kind="ExternalOutput")

    with tile.TileContext(nc) as tc:
        with tc.tile_pool(name="consts", bufs=1) as consts, \
             tc.tile_pool(name="work", bufs=2) as work, \
             tc.tile_pool(name="fin", bufs=4) as finp, \
             tc.tile_pool(name="brp", bufs=2) as brp, \
             tc.tile_pool(name="a2p", bufs=4) as a2p, \
             tc.tile_pool(name="atp", bufs=2) as atp, \
             tc.tile_pool(name="ps_t", bufs=2, space="PSUM") as ps_t, \
             tc.tile_pool(name="ps_c", bufs=2, space="PSUM") as ps_c:
            ov = od[:].rearrange("p (i c) -> p i c", i=M)
            # input DMAs spread over four sequencers/queues
            xc_st = consts.tile([P, M * 6], F32)
            nc.sync.dma_start(out=xc_st[:], in_=xd[:])
            ident = consts.tile([P, P], BF16)
            nc.scalar.dma_start(out=ident[:], in_=idd[:])
            s2_sb = consts.tile([P, 2, WID], BF16)
            nc.gpsimd.dma_start(out=s2_sb[:, 0, :], in_=s2ad[:])
            nc.gpsimd.dma_start(out=s2_sb[:, 1, :], in_=s2bd[:])

            # HAM warm-up: junk matmuls on the otherwise idle PE while the
            # front (DMAs, DVE H/E/kron) runs.  fp32 (slow path) on purpose.
            for _ in range(N_WARM):
                wps = ps_t.tile([P, 4, P], F32, tag="tp")
                nc.tensor.matmul(wps[0:96, 0, 0:96],
                                 lhsT=xc_st[:, 0:96], rhs=xc_st[:, 0:96],
                                 start=True, stop=True)

            invt = -neg3invt / 3.0
            xv = xc_st[:, 0:M * 6].rearrange("p (i f) -> p i f", i=M)

            # --- front per 8-slot half: H, E, pp ---
            H = work.tile([P, M, 6, 4], F32, tag="H")
            E = work.tile([P, M, 6, 4], F32, tag="E")
            pp = work.tile([P, M, 3, 16], BF16, tag="pp")
            for hf in range(2):
                sl = slice(hf * 8, hf * 8 + 8)
                nc.vector.tensor_scalar(out=H[:, sl, :, 0], in0=xv[:, sl],
                                        scalar1=0.0, scalar2=neg3invt,
                                        op0=mybir.AluOpType.max,
                                        op1=mybir.AluOpType.mult)
                for j in range(1, 4):
                    nc.vector.scalar_tensor_tensor(
                        out=H[:, sl, :, j], in0=xv[:, sl],
                        scalar=float(j) * invt, in1=H[:, sl, :, 0],
                        op0=mybir.AluOpType.mult, op1=mybir.AluOpType.add)
                nc.scalar.activation(
                    E[:, sl].rearrange("p i f j -> p (i f j)"),
                    H[:, sl].rearrange("p i f j -> p (i f j)"),
                    mybir.ActivationFunctionType.Exp)
                E2 = E[:, sl].rearrange("p i (g t) j -> p i g t j", t=2)
                nc.vector.tensor_mul(
                    pp[:, sl].rearrange("p i f (a b) -> p i f a b", a=4),
                    E2[:, :, :, 0, :, None].broadcast_to((P, 8, 3, 4, 4)),
                    E2[:, :, :, 1, None, :].broadcast_to((P, 8, 3, 4, 4)))
            pp01 = pp[:, :, 0, :]
            pp23 = pp[:, :, 1, :]
            p45 = pp[:, :, 2, :]

            def emit_a2_tp(bg):
                base = bg * QS
                slp = slice(base, base + QS)
                A2 = a2p.tile([P, QS, 16, 16], BF16, tag="a2", name=f"A2{bg}")
                if bg == 0:
                    # fast lane: direct 1x kron, no Brep hop
                    nc.vector.tensor_mul(
                        A2[:],
                        pp23[:, slp, :, None].broadcast_to((P, QS, 16, 16)),
                        pp01[:, slp, None, :].broadcast_to((P, QS, 16, 16)))
                else:
                    Br = brp.tile([P, QS, 16, 16], BF16, tag="br",
                                  name=f"Br{bg}")
                    nc.scalar.copy(out=Br[:],
                                   in_=pp23[:, slp, :, None].broadcast_to(
                                       (P, QS, 16, 16)))
                    nc.vector.tensor_mul(
                        A2[:], Br[:],
                        pp01[:, slp, None, :].broadcast_to((P, QS, 16, 16)))
                A2v = A2[:].rearrange("p s c a -> p s (c a)")
                tp = ps_t.tile([P, 2 * QS, P], BF16, tag="tp", name=f"tp{bg}")
                for s in range(QS):
                    for k in range(2):
                        nc.tensor.transpose(tp[:, 2 * s + k, :],
                                            A2v[:, s, k * P:(k + 1) * P],
                                            ident[:])
                return tp

            def emit_back(bg, tp):
                base = bg * QS
                slp = slice(base, base + QS)
                first = bg == 0
                last = bg == NG - 1
                at2 = atp.tile([P, 2 * QS, P], BF16, tag="at", name=f"at{bg}")
                if first:
                    # bf16 packed PSUM->SBUF copy dual-pumps on DVE
                    nc.vector.tensor_copy(out=at2[:], in_=tp[:])
                else:
                    nc.scalar.copy(out=at2[:], in_=tp[:])
                if last:
                    # tail: per 2-slot half chain, top priority, so the very
                    # last rows take the shortest path through DVE
                    for h2 in range(2):
                        cpp = ps_c.tile([P, 2, 256], F32, tag="cph",
                                        name=f"cp{bg}_{h2}")
                        for s2i in range(2):
                            s = 2 * h2 + s2i
                            nc.tensor.matmul(cpp[:, s2i, 0:WID],
                                             lhsT=at2[:, 2 * s, :],
                                             rhs=s2_sb[:, 0, :], start=True,
                                             stop=False)
                            nc.tensor.matmul(cpp[:, s2i, 0:WID],
                                             lhsT=at2[:, 2 * s + 1, :],
                                             rhs=s2_sb[:, 1, :], start=False,
                                             stop=True)
                        cv = cpp[:, :, 0:WID].rearrange(
                            "p s (c v) -> p s c v", c=NBLK)
                        sl2 = slice(base + 2 * h2, base + 2 * h2 + 2)
                        with tc.high_priority(offset=100):
                            D4 = finp.tile([P, 2, NBLK, 16], F32, tag="Dl",
                                           name=f"D{bg}_{h2}")
                            nc.vector.tensor_mul(
                                D4[:], cv,
                                p45[:, sl2, None, :].broadcast_to(
                                    (P, 2, NBLK, 16)))
                            Og = finp.tile([P, 2, NBLK], F32, tag="Ol",
                                           name=f"Og{bg}_{h2}")
                            nc.vector.tensor_reduce(Og[:], D4[:],
                                                    axis=mybir.AxisListType.X,
                                                    op=mybir.AluOpType.add)
                            zr = finp.tile([P, 2, 1], F32, tag="zrl",
                                           name=f"zr{bg}_{h2}")
                            nc.vector.reciprocal_approx_fast(zr[:, :, 0],
                                                             Og[:, :, 10])
                            Of = finp.tile([P, 2, 10], F32, tag="Ofl",
                                           name=f"Of{bg}_{h2}")
                            nc.vector.tensor_mul(
                                Of[:], Og[:, :, 0:10],
                                zr[:].broadcast_to((P, 2, 10)))
                            row0 = base + 2 * h2
                            nc.sync.dma_start(out=ov[:, row0:row0 + 2, :],
                                              in_=Of[:])
                    return
                cpp = ps_c.tile([P, QS, 256], F32, tag="cp", name=f"cp{bg}")
                for s in range(QS):
                    nc.tensor.matmul(cpp[:, s, 0:WID], lhsT=at2[:, 2 * s, :],
                                     rhs=s2_sb[:, 0, :], start=True,
                                     stop=False)
                    nc.tensor.matmul(cpp[:, s, 0:WID],
                                     lhsT=at2[:, 2 * s + 1, :],
                                     rhs=s2_sb[:, 1, :], start=False,
                                     stop=True)
                cv = cpp[:, :, 0:WID].rearrange("p s (c v) -> p s c v",
                                                c=NBLK)
                Og = finp.tile([P, QS, NBLK], F32, tag="O", name=f"Og{bg}")
                if first:
                    # head: D + reduce-16 + finalize all on DVE under a
                    # priority boost -- fewest cross-engine hops
                    with tc.high_priority(offset=60):
                        D4 = finp.tile([P, QS, NBLK, 16], F32, tag="D",
                                       name=f"D{bg}")
                        nc.vector.tensor_mul(
                            D4[:], cv,
                            p45[:, slp, None, :].broadcast_to(
                                (P, QS, NBLK, 16)))
                        nc.vector.tensor_reduce(Og[:], D4[:],
                                                axis=mybir.AxisListType.X,
                                                op=mybir.AluOpType.add)
                        zr = finp.tile([P, QS, 1], F32, tag="zr",
                                       name=f"zr{bg}")
                        nc.vector.reciprocal_approx_fast(zr[:, :, 0],
                                                         Og[:, :, 10])
                        Of = finp.tile([P, QS, 10], F32, tag="Of",
                                       name=f"Of{bg}")
                        nc.gpsimd.tensor_mul(Of[:], Og[:, :, 0:10],
                                             zr[:].broadcast_to((P, QS, 10)))
                        nc.scalar.dma_start(out=ov[:, base:base + QS, :],
                                            in_=Of[:])
                else:
                    D4 = finp.tile([P, QS, NBLK, 16], F32, tag="D",
                                   name=f"D{bg}")
                    nc.vector.tensor_mul(
                        D4[:], cv,
                        p45[:, slp, None, :].broadcast_to((P, QS, NBLK, 16)))
                    Dh = finp.tile([P, QS, NBLK, 8], F32, tag="Dh",
                                   name=f"Dh{bg}")
                    nc.gpsimd.tensor_add(Dh[:], D4[:, :, :, 0:8],
                                         D4[:, :, :, 8:16])
                    with tc.high_priority(offset=20):
                        nc.vector.tensor_reduce(Og[:], Dh[:],
                                                axis=mybir.AxisListType.X,
                                                op=mybir.AluOpType.add)
                        zr = finp.tile([P, QS, 1], F32, tag="zr",
                                       name=f"zr{bg}")
                        nc.vector.reciprocal_approx_fast(zr[:, :, 0],
                                                         Og[:, :, 10])
                        Of = finp.tile([P, QS, 10], F32, tag="Of",
                                       name=f"Of{bg}")
                        if bg == 1:
                            nc.gpsimd.tensor_mul(
                                Of[:], Og[:, :, 0:10],
                                zr[:].broadcast_to((P, QS, 10)))
                        else:
                            nc.vector.tensor_mul(
                                Of[:], Og[:, :, 0:10],
                                zr[:].broadcast_to((P, QS, 10)))
                        nc.scalar.dma_start(out=ov[:, base:base + QS, :],
                                            in_=Of[:])

            # group 0's full chain first (lowest latency to first output),
            # then the remaining A2/transposes (PE streams them while ACT
            # stages at2), then the remaining back halves in order.
            tp0 = emit_a2_tp(0)
            emit_back(0, tp0)
            tps = {bg: emit_a2_tp(bg) for bg in range(1, NG)}
            for bg in range(1, NG):
                emit_back(bg, tps[bg])
    nc.compile()
    return nc


def prep_inputs(x, cuts, leaf_score, temperature):
    """Host-side parameter prep (tiny). Returns (in_maps, invt)."""
    import ml_dtypes
    x = np.ascontiguousarray(np.asarray(x, dtype=np.float32))
    cuts = np.asarray(cuts, dtype=np.float32)
    leaf_score = np.asarray(leaf_score, dtype=np.float32)
    invt = 1.0 / float(np.asarray(temperature).reshape(-1)[0])

    sc = np.sort(cuts, axis=1)
    bias = np.cumsum(np.concatenate([np.zeros((6, 1), np.float64), -sc],
                                    axis=1, dtype=np.float64), axis=1)  # [6,4]
    ebt = np.exp(bias * invt)                                           # [6,4]
    c0123 = np.einsum('a,b,c,d->abcd', ebt[0], ebt[1], ebt[2],
                      ebt[3]).reshape(256)
    c45 = np.einsum('a,b->ab', ebt[4], ebt[5]).reshape(16)
    xs = x.reshape(N_CORES, P, M * 6)

    s2 = np.zeros((256, WID), np.float64)
    s2[:, :160] = leaf_score.reshape(256, 16, 10).transpose(0, 2, 1).reshape(
        256, 160)
    s2[:, 160:] = 1.0
    s2 = s2 * c0123[:, None] * np.tile(c45, NBLK)[None, :]
    # device A2 kron is in (cd, ab) order; permute rows to match
    s2 = s2.reshape(16, 16, WID).transpose(1, 0, 2).reshape(256, WID)
    s2 = s2.reshape(2, P, WID).astype(ml_dtypes.bfloat16)
    ident = np.eye(P, dtype=ml_dtypes.bfloat16)

    common = {"id": np.ascontiguousarray(ident),
              "s2a": np.ascontiguousarray(s2[0]),
              "s2b": np.ascontiguousarray(s2[1])}
    in_maps = [dict(common, xc=np.ascontiguousarray(xs[i]))
               for i in range(N_CORES)]
    return in_maps, invt


_CACHE = {}


def kernel(x, cuts, leaf_score, temperature):
    in_maps, invt = prep_inputs(x, cuts, leaf_score, temperature)
    key = ("nc", float(invt))
    if key not in _CACHE:
        _CACHE[key] = _build_nc(-3.0 * invt)
        _CACHE["nc"] = _CACHE[key]
    nc = _CACHE[key]
    res = run_bass_kernel_spmd(nc, in_maps, list(range(N_CORES))).results
    out = np.concatenate([r["o"].reshape(BC, 10) for r in res], axis=0)
    return out.astype(np.float32)
